# revision 14
# baseline (speedup 1.0000x reference)
"""Trainium2 Bass kernel for nn_CrossAttention (single-query cross attention).

Reference computation (B=4, C=64, H=W=128, heads h=64, dim_head d=64,
inner=4096, HW=16384):
    x[b, j, c]   = fimg[b, c, j]                       (j indexes H*W)
    q[b, h, d]   = sum_e fpsf[b, e] Wq[h*64+d, e]
    k[b, j, h, d]= sum_c x[b, j, c] Wk[h*64+d, c]
    out[b, h, j] = scale * sum_d q[b,h,d] k[b,j,h,d]

Because there is a single query per (batch, head), the attention collapses:
    W2[b, h, c]  = scale * sum_d q[b,h,d] Wk[h*64+d, c]      (tiny)
    out[b, h, j] = sum_c W2[b,h,c] fimg[b, c, j]
a 64x FLOP reduction vs materializing k.

Sharding: the j (H*W = 16384) axis is split across the 8 cores (2048 each).
Every core redundantly computes W2 (it needs all heads for its output).

Profile-driven history (exec_time best-of-3):
  v1 29.0us; v2 26.6us (DMA issues 11->6, weights-before-fimg, bf16 out);
  v4-v9 26.9-31.8 (chunking/warm-up experiments: more DMAs = worse).
  v10: the key DMA fact is that SDMA engine k serves a fixed set of 8
  SBUF partitions, so a [64, N] transfer only engages half the 16
  engines (~185 GB/s instead of ~370). All transfers are therefore
  [128, N]; the weight halves destined for matmul operands (which must
  sit at partition 0 - offset-64 operands hard-crash the PE, verified)
  are lowered 64->0 by the otherwise-idle GpSimd engine.

Device layouts (prepared host-side; host does LAYOUT only, no math):
  Wq2    [128, 2052] bf16: rows 0:64  = [fpsf.T | Wq.T cols 0:2048]
                           rows 64:128= [fpsf.T | Wq.T cols 2048:4096]
  Wk2    [128, 2048] bf16: Wk_nat[d, 64h+c] = Wk[64h+d, c];
                           rows 0:64 = cols 0:2048 (head pairs 0-15),
                           rows 64:128 = cols 2048:4096 (pairs 16-31)
  fimg_s [128, 4096] bf16: rows b%2*64+c, cols 2048*(b//2) + local j
  out    [128, 4096] bf16: rows b%2*64+h, cols 2048*(b//2) + local j

Device compute per core:
  warm: 8 junk 512-col matmuls flip the PE HAM clock gate to 8/8
     (2.4GHz vs cold 1.2GHz) while the weight DMAs land.
  GpSimd: wq_hi[64, 2048] <- Wq2[64:128, 4:], wk_hi <- Wk2[64:128, :]
  A: 32 matmuls (16 lo + 16 hi)  q2T chunk [128, 4] = WqT_chunk.T @ fpsfT
     -> q2T psum [128, 128]: rows d+64*(h%2), cols 4*(h//2)+b
  copy: psum halves -> SBUF bf16 q2e/q2o [64, 128] (scale folded),
     DVE + ACT in parallel.
  B: 64 matmuls, head-pair p loads the [64, 128] pair block (128-col
     stationary -> compiler FWL) twice: rhs q2e[:, 4p:4p+4] ->
     w2a[:, 4p:4p+4] (rows 0:64 valid), rhs q2o -> w2b (rows 64:128).
  Assembly: per batch-pair q, block-diag lhsT bd_q [128, 128] bf16:
     bd_q[64*half + c, 64*half + h] = W2[2q+half, h, c]
     (bd0's copies on DVE, bd1's on ACT, in parallel)
  Big: 8 matmuls [128, 512] = bd_q.T @ fimg cols; psum -> bf16 staging
     [128, 2048] per q (vector/scalar alternate); out DMA per q.
"""

import sys
import types

import numpy as np
import ml_dtypes

# antenv.axon_hooks is absent in this image; bass_utils imports it when
# tracing. Register a minimal stand-in before importing concourse.
if "antenv.axon_hooks" not in sys.modules:
    try:
        import antenv  # noqa: F401

        _hooks = types.ModuleType("antenv.axon_hooks")
        _hooks._hook = None

        def _set_hook(h):
            _hooks._hook = h

        _hooks.set_axon_ntff_profile_hook = _set_hook
        _hooks.get_axon_ntff_profile_hook = lambda: _hooks._hook
        sys.modules["antenv.axon_hooks"] = _hooks
        try:
            from trn_agent_boot.trn_boot import _ntff_profile_via_ctypes

            _set_hook(_ntff_profile_via_ctypes("/opt/axon/libaxon_pjrt.so"))
        except Exception:
            pass
    except ImportError:
        pass

import concourse.bass as bass  # noqa: E402
import concourse.mybir as mybir  # noqa: E402
import concourse.tile as tile  # noqa: E402
from concourse import bacc  # noqa: E402
from concourse.bass_utils import run_bass_kernel_spmd  # noqa: E402

N_CORES = 8
B, C, H, W = 4, 64, 128, 128
HEADS, DIM_HEAD = 64, 64
HW = H * W
JS = HW // N_CORES  # 2048 j-positions per core
SCALE = DIM_HEAD ** -0.5
F32 = mybir.dt.float32
BF16 = mybir.dt.bfloat16
NPBF16 = ml_dtypes.bfloat16

_compiled = None  # cache (nc) across calls


def _build():
    nc = bacc.Bacc("TRN2", target_bir_lowering=False, debug=False,
                   num_devices=N_CORES)

    fimg_d = nc.dram_tensor("fimg_s", [128, 2 * JS], BF16, kind="ExternalInput")
    wq2_d = nc.dram_tensor("Wq2", [128, 2052], BF16, kind="ExternalInput")
    wk2_d = nc.dram_tensor("Wk2", [128, 2048], BF16, kind="ExternalInput")
    out_d = nc.dram_tensor("out", [128, 2 * JS], BF16, kind="ExternalOutput")

    with tile.TileContext(nc) as tc:
        with (
            tc.tile_pool(name="weights", bufs=1) as wpool,
            tc.tile_pool(name="img", bufs=1) as ipool,
            tc.tile_pool(name="small_ps", bufs=1, space="PSUM") as spsum,
            tc.tile_pool(name="big_ps", bufs=5, space="PSUM") as bpsum,
            tc.tile_pool(name="ostage", bufs=1) as opool,
        ):
            # Input DMAs in priority order on the Sync HWDGE ring; all
            # [128, N] so every SDMA engine carries an equal share.
            wq2 = wpool.tile([128, 2052], BF16, tag="wq2")
            nc.sync.dma_start(wq2[:], wq2_d.ap()[:])
            fpsfT = wq2[0:64, 0:4]
            wk2 = wpool.tile([128, 2048], BF16, tag="wk2")
            nc.sync.dma_start(wk2[:], wk2_d.ap()[:])
            imgs = ipool.tile([128, 2 * JS], BF16, tag="img")
            nc.sync.dma_start(imgs[:], fimg_d.ap()[:])

            # GpSimd (idle otherwise) lowers the upper-half weight rows
            # to base partition 0 for use as matmul operands.
            wq_hi = wpool.tile([64, 2048], BF16, tag="wq_hi")
            nc.gpsimd.tensor_copy(wq_hi[:], wq2[64:128, 4:2052])
            wk_hi = wpool.tile([64, 2048], BF16, tag="wk_hi")
            nc.gpsimd.tensor_copy(wk_hi[:], wk2[64:128, :])

            # Warm-up: ~3.4us of junk matmuls flips the PE HAM clock gate
            # to 8/8 (2.4GHz) before the real matmuls issue.
            warm = wpool.tile([128, 640], BF16, tag="warm")
            nc.vector.memset(warm[:], 0.0)
            for _ in range(8):
                wps = bpsum.tile([128, 512], F32, tag="mm_ps")
                nc.tensor.matmul(wps[:], warm[:, 0:128], warm[:, 128:640],
                                 start=True, stop=True)

            # A: q2T[d + 64*(h%2), 4*(h//2)+b] = q[b, h, d] (unscaled)
            q2T_ps = spsum.tile([128, 128], F32, tag="q2T_ps")
            for p in range(16):
                nc.tensor.matmul(
                    q2T_ps[:, 4 * p:4 * p + 4],
                    wq2[0:64, 4 + 128 * p:4 + 128 * p + 128],
                    fpsfT,
                    start=True, stop=True,
                )
            for p in range(16):
                nc.tensor.matmul(
                    q2T_ps[:, 64 + 4 * p:64 + 4 * p + 4],
                    wq_hi[:, 128 * p:128 * p + 128],
                    fpsfT,
                    start=True, stop=True,
                )
            # Scale folded into the PSUM->SBUF copies; halves go to
            # disjoint tiles on different engines so they run in parallel.
            q2e = wpool.tile([64, 128], BF16, tag="q2e")
            q2o = wpool.tile([64, 128], BF16, tag="q2o")
            nc.vector.tensor_scalar_mul(q2e[:], q2T_ps[0:64, :], SCALE)
            nc.scalar.mul(q2o[:], q2T_ps[64:128, :], SCALE)

            # B: pair p stationary = [64, 128] pair block (cols 0:64 =
            # head 2p, 64:128 = head 2p+1). Two matmuls reuse it:
            #   w2a[c, 4p+b]      = W2[b, 2p, c]      (rows 64: garbage)
            #   w2b[64+c, 4p+b]   = W2[b, 2p+1, c]    (rows <64: garbage)
            w2a = spsum.tile([128, 128], F32, tag="w2a")
            w2b = spsum.tile([128, 128], F32, tag="w2b")
            for p in range(32):
                if p < 16:
                    lhsT = wk2[0:64, 128 * p:128 * p + 128]
                else:
                    lhsT = wk_hi[:, 128 * (p - 16):128 * (p - 16) + 128]
                nc.tensor.matmul(w2a[:, 4 * p:4 * p + 4], lhsT,
                                 q2e[:, 4 * p:4 * p + 4], start=True, stop=True)
                nc.tensor.matmul(w2b[:, 4 * p:4 * p + 4], lhsT,
                                 q2o[:, 4 * p:4 * p + 4], start=True, stop=True)

            # Assembly: bd_q[64*half + c, 64*half + h] = W2[2q+half, h, c]
            # bd0's four copies on DVE, bd1's on ACT (parallel; bd0 gates
            # the first big chunks and lands early).
            bds = []
            for q in range(2):
                bd = wpool.tile([128, 128], BF16, tag=f"bd{q}")
                nc.vector.memset(bd[:], 0.0)
                for half in range(2):
                    b = 2 * q + half
                    for parity in range(2):
                        dst = bd[64 * half:64 * half + 64,
                                 64 * half + parity:64 * half + 64:2]
                        src = (w2a if parity == 0 else w2b)[
                            64 * parity:64 * parity + 64, b:128:4]
                        if q == 0:
                            nc.vector.tensor_copy(dst, src)
                        else:
                            nc.scalar.copy(dst, src)
                bds.append(bd)

            # Big: out rows pair q = bd_q.T @ img_q, in 512-col chunks.
            # PSUM -> bf16 staging (vector/scalar alternate), one output
            # DMA per q on the Sync ring (idle after the input issues).
            for q in range(2):
                ot = opool.tile([128, JS], BF16, tag=f"ot{q}")
                for k in range(4):
                    ps = bpsum.tile([128, 512], F32, tag="mm_ps")
                    nc.tensor.matmul(
                        ps[:], bds[q][:],
                        imgs[:, JS * q + 512 * k:JS * q + 512 * k + 512],
                        start=True, stop=True,
                    )
                    dst = ot[:, 512 * k:512 * k + 512]
                    if k % 2 == 0:
                        nc.vector.tensor_copy(dst, ps[:])
                    else:
                        nc.scalar.copy(dst, ps[:])
                nc.sync.dma_start(
                    out_d.ap()[:, JS * q:JS * (q + 1)], ot[:])

    nc.compile()
    return nc


def _prep_inputs(fpsf, fimg, Wq, Wk):
    fpsf = np.ascontiguousarray(fpsf, dtype=np.float32)
    fimg = np.ascontiguousarray(fimg, dtype=np.float32)
    Wq = np.ascontiguousarray(Wq, dtype=np.float32)
    Wk = np.ascontiguousarray(Wk, dtype=np.float32)

    fpsfT = fpsf.T.astype(NPBF16)
    WqT = Wq.T.astype(NPBF16)  # [64, 4096]
    Wq2 = np.empty((128, 2052), NPBF16)
    Wq2[0:64, 0:4] = fpsfT
    Wq2[64:128, 0:4] = fpsfT
    Wq2[0:64, 4:2052] = WqT[:, 0:2048]
    Wq2[64:128, 4:2052] = WqT[:, 2048:4096]

    # Wk_nat[d, 64h+c] = Wk[64h+d, c]
    Wk_nat = np.ascontiguousarray(
        Wk.reshape(64, 64, 64).transpose(1, 0, 2).reshape(64, 4096)
    ).astype(NPBF16)
    Wk2 = np.empty((128, 2048), NPBF16)
    Wk2[0:64, :] = Wk_nat[:, 0:2048]
    Wk2[64:128, :] = Wk_nat[:, 2048:4096]

    fimg_f = fimg.reshape(B, C, HW).astype(NPBF16)
    in_maps = []
    for i in range(N_CORES):
        sh = np.ascontiguousarray(
            fimg_f[:, :, JS * i:JS * (i + 1)]).reshape(2, 128, JS)
        sh = np.ascontiguousarray(
            sh.transpose(1, 0, 2).reshape(128, 2 * JS))
        in_maps.append({
            "fimg_s": sh,
            "Wq2": Wq2,
            "Wk2": Wk2,
        })
    return in_maps


def kernel(fpsf, fimg, Wq, Wk):
    global _compiled
    if _compiled is None:
        _compiled = _build()
    nc = _compiled

    in_maps = _prep_inputs(fpsf, fimg, Wq, Wk)
    res = run_bass_kernel_spmd(nc, in_maps, core_ids=list(range(N_CORES)))

    out = np.empty((B, HEADS, HW), dtype=np.float32)
    for i in range(N_CORES):
        o = res.results[i]["out"]  # [128, 2*JS] bf16
        o = o.reshape(128, 2, JS).transpose(1, 0, 2).reshape(B, HEADS, JS)
        out[:, :, JS * i:JS * (i + 1)] = o.astype(np.float32)
    return out.reshape(B, C, H, W)


if __name__ == "__main__":
    rng = np.random.default_rng(0)
    ins = {
        "fpsf": rng.standard_normal((B, C), dtype=np.float32),
        "fimg": rng.standard_normal((B, C, H, W), dtype=np.float32),
        "Wq": (rng.standard_normal((4096, C), dtype=np.float32) * 0.05),
        "Wk": (rng.standard_normal((4096, C), dtype=np.float32) * 0.05),
    }
    out = kernel(**ins)
    print("out", out.shape, out.dtype, float(np.abs(out).max()))


# revision 18
# speedup vs baseline: 1.3513x; 1.3513x over previous
"""Trainium2 Bass kernel for nn_CrossAttention (single-query cross attention).

Reference computation (B=4, C=64, H=W=128, heads h=64, dim_head d=64,
inner=4096, HW=16384):
    x[b, j, c]   = fimg[b, c, j]                       (j indexes H*W)
    q[b, h, d]   = sum_e fpsf[b, e] Wq[h*64+d, e]
    k[b, j, h, d]= sum_c x[b, j, c] Wk[h*64+d, c]
    out[b, h, j] = scale * sum_d q[b,h,d] k[b,j,h,d]

Because there is a single query per (batch, head), the attention collapses:
    W2[b, h, c]  = scale * sum_d q[b,h,d] Wk[h*64+d, c]      (tiny)
    out[b, h, j] = sum_c W2[b,h,c] fimg[b, c, j]
a 64x FLOP reduction vs materializing k.

Sharding: the j (H*W = 16384) axis is split across the 8 cores (2048 each).
Every core redundantly computes W2 (it needs all heads for its output).

Profile-driven history (exec_time best-of-3):
  v1 29.0us; v2 26.6us (DMA issues 11->6, weights-before-fimg, bf16 out);
  v4-v9 26.9-31.8 (chunking/warm-up experiments: more DMAs = worse).
  v10: the key DMA fact is that SDMA engine k serves a fixed set of 8
  SBUF partitions, so a [64, N] transfer only engages half the 16
  engines (~185 GB/s instead of ~370). All transfers are therefore
  [128, N]; the weight halves destined for matmul operands (which must
  sit at partition 0 - offset-64 operands hard-crash the PE, verified)
  are lowered 64->0 by the otherwise-idle GpSimd engine.

Device layouts (prepared host-side; host does LAYOUT only, no math):
  Wq2    [128, 2052] bf16: rows 0:64  = [fpsf.T | Wq.T cols 0:2048]
                           rows 64:128= [fpsf.T | Wq.T cols 2048:4096]
  Wk2    [128, 2048] bf16: Wk_nat[d, 64h+c] = Wk[64h+d, c];
                           rows 0:64 = cols 0:2048 (head pairs 0-15),
                           rows 64:128 = cols 2048:4096 (pairs 16-31)
  fimg_s [128, 4096] bf16: rows b%2*64+c, cols 2048*(b//2) + local j
  out    [128, 4096] bf16: rows b%2*64+h, cols 2048*(b//2) + local j

Device compute per core:
  warm: 8 junk 512-col matmuls flip the PE HAM clock gate to 8/8
     (2.4GHz vs cold 1.2GHz) while the weight DMAs land.
  GpSimd: wq_hi[64, 2048] <- Wq2[64:128, 4:], wk_hi <- Wk2[64:128, :]
  A: 32 matmuls (16 lo + 16 hi)  q2T chunk [128, 4] = WqT_chunk.T @ fpsfT
     -> q2T psum [128, 128]: rows d+64*(h%2), cols 4*(h//2)+b
  copy: psum halves -> SBUF bf16 q2e/q2o [64, 128] (scale folded),
     DVE + ACT in parallel.
  B: 64 matmuls, head-pair p loads the [64, 128] pair block (128-col
     stationary -> compiler FWL) twice: rhs q2e[:, 4p:4p+4] ->
     w2a[:, 4p:4p+4] (rows 0:64 valid), rhs q2o -> w2b (rows 64:128).
  Assembly: per batch-pair q, block-diag lhsT bd_q [128, 128] bf16:
     bd_q[64*half + c, 64*half + h] = W2[2q+half, h, c]
     (bd0's copies on DVE, bd1's on ACT, in parallel)
  Big: 8 matmuls [128, 512] = bd_q.T @ fimg cols; psum -> bf16 staging
     [128, 2048] per q (vector/scalar alternate); out DMA per q.
"""

import sys
import types

import numpy as np
import ml_dtypes

# antenv.axon_hooks is absent in this image; bass_utils imports it when
# tracing. Register a minimal stand-in before importing concourse.
if "antenv.axon_hooks" not in sys.modules:
    try:
        import antenv  # noqa: F401

        _hooks = types.ModuleType("antenv.axon_hooks")
        _hooks._hook = None

        def _set_hook(h):
            _hooks._hook = h

        _hooks.set_axon_ntff_profile_hook = _set_hook
        _hooks.get_axon_ntff_profile_hook = lambda: _hooks._hook
        sys.modules["antenv.axon_hooks"] = _hooks
        try:
            from trn_agent_boot.trn_boot import _ntff_profile_via_ctypes

            _set_hook(_ntff_profile_via_ctypes("/opt/axon/libaxon_pjrt.so"))
        except Exception:
            pass
    except ImportError:
        pass

import concourse.bass as bass  # noqa: E402
import concourse.mybir as mybir  # noqa: E402
import concourse.tile as tile  # noqa: E402
from concourse import bacc  # noqa: E402
from concourse.bass_utils import run_bass_kernel_spmd  # noqa: E402

N_CORES = 8
B, C, H, W = 4, 64, 128, 128
HEADS, DIM_HEAD = 64, 64
HW = H * W
JS = HW // N_CORES  # 2048 j-positions per core
SCALE = DIM_HEAD ** -0.5
F32 = mybir.dt.float32
BF16 = mybir.dt.bfloat16
NPBF16 = ml_dtypes.bfloat16

_compiled = None  # cache (nc) across calls


def _build():
    nc = bacc.Bacc("TRN2", target_bir_lowering=False, debug=False,
                   num_devices=N_CORES)

    fimg_d = nc.dram_tensor("fimg_s", [128, 2 * JS], BF16, kind="ExternalInput")
    wqf_d = nc.dram_tensor("WqF", [64, 4100], BF16, kind="ExternalInput")
    wk2_d = nc.dram_tensor("Wk2", [128, 2048], BF16, kind="ExternalInput")
    out_d = nc.dram_tensor("out", [128, 2 * JS], BF16, kind="ExternalOutput")

    with tile.TileContext(nc) as tc:
        with (
            tc.tile_pool(name="weights", bufs=1) as wpool,
            tc.tile_pool(name="img", bufs=1) as ipool,
            tc.tile_pool(name="small_ps", bufs=1, space="PSUM") as spsum,
            tc.tile_pool(name="big_ps", bufs=5, space="PSUM") as bpsum,
            tc.tile_pool(name="ostage", bufs=1) as opool,
        ):
            # Input DMAs in priority order on the Sync HWDGE ring. wqf is
            # [64, N] (only 8 of 16 SDMA engines, ~185GB/s) but it's first
            # and off the critical tail; wk2/fimg are [128, N] full-rate.
            wqf = wpool.tile([64, 4100], BF16, tag="wqf")
            nc.sync.dma_start(wqf[:], wqf_d.ap()[:])
            fpsfT = wqf[:, 0:4]
            wqT = wqf[:, 4:4100]
            wk2 = wpool.tile([128, 2048], BF16, tag="wk2")
            nc.sync.dma_start(wk2[:], wk2_d.ap()[:])
            imgs = ipool.tile([128, 2 * JS], BF16, tag="img")
            nc.sync.dma_start(imgs[:, 0:JS], fimg_d.ap()[:, 0:JS])
            nc.sync.dma_start(imgs[:, JS:2 * JS], fimg_d.ap()[:, JS:2 * JS])

            # Warm-up: ~3.4us of junk matmuls flips the PE HAM clock gate
            # to 8/8 (2.4GHz) before the real matmuls issue.
            warm = wpool.tile([128, 640], BF16, tag="warm")
            nc.vector.memset(warm[:], 0.0)
            for _ in range(8):
                wps = bpsum.tile([128, 512], F32, tag="mm_ps")
                nc.tensor.matmul(wps[:], warm[:, 0:128], warm[:, 128:640],
                                 start=True, stop=True)

            # A: q2T[d + 64*(h%2), 4*(h//2)+b] = q[b, h, d] (unscaled)
            q2T_ps = spsum.tile([128, 128], F32, tag="q2T_ps")
            for p in range(32):
                nc.tensor.matmul(
                    q2T_ps[:, 4 * p:4 * p + 4],
                    wqT[:, 128 * p:128 * p + 128],
                    fpsfT,
                    start=True, stop=True,
                )
            # Scale folded into the PSUM->SBUF copies; halves go to
            # disjoint tiles on different engines so they run in parallel.
            q2e = wpool.tile([64, 128], BF16, tag="q2e")
            q2o = wpool.tile([64, 128], BF16, tag="q2o")
            nc.vector.tensor_scalar_mul(q2e[:], q2T_ps[0:64, :], SCALE)
            nc.scalar.mul(q2o[:], q2T_ps[64:128, :], SCALE)

            # Lower the upper half of wk2 to base partition 0 (matmul
            # operands at partition offset 64 hard-crash the PE). Halves
            # on DVE + ACT, emitted after the q copies so those (which
            # gate B's first half) keep queue priority.
            wk_hi = wpool.tile([64, 2048], BF16, tag="wk_hi")
            nc.vector.tensor_copy(wk_hi[:, 0:1024], wk2[64:128, 0:1024])
            nc.scalar.copy(wk_hi[:, 1024:2048], wk2[64:128, 1024:2048])

            # B: pair p stationary = [64, 128] pair block (cols 0:64 =
            # head 2p, 64:128 = head 2p+1). Two matmuls reuse it:
            #   w2a[c, 4p+b]      = W2[b, 2p, c]      (rows 64: garbage)
            #   w2b[64+c, 4p+b]   = W2[b, 2p+1, c]    (rows <64: garbage)
            w2a = spsum.tile([128, 128], F32, tag="w2a")
            w2b = spsum.tile([128, 128], F32, tag="w2b")
            for p in range(32):
                if p < 16:
                    lhsT = wk2[0:64, 128 * p:128 * p + 128]
                else:
                    lhsT = wk_hi[:, 128 * (p - 16):128 * (p - 16) + 128]
                nc.tensor.matmul(w2a[:, 4 * p:4 * p + 4], lhsT,
                                 q2e[:, 4 * p:4 * p + 4], start=True, stop=True)
                nc.tensor.matmul(w2b[:, 4 * p:4 * p + 4], lhsT,
                                 q2o[:, 4 * p:4 * p + 4], start=True, stop=True)

            # Assembly: bd_q[64*half + c, 64*half + h] = W2[2q+half, h, c]
            # bd0's four copies on DVE, bd1's on ACT (parallel; bd0 gates
            # the first big chunks and lands early).
            bds = []
            for q in range(2):
                bd = wpool.tile([128, 128], BF16, tag=f"bd{q}")
                nc.vector.memset(bd[:], 0.0)
                for half in range(2):
                    b = 2 * q + half
                    for parity in range(2):
                        dst = bd[64 * half:64 * half + 64,
                                 64 * half + parity:64 * half + 64:2]
                        src = (w2a if parity == 0 else w2b)[
                            64 * parity:64 * parity + 64, b:128:4]
                        if q == 0:
                            nc.vector.tensor_copy(dst, src)
                        else:
                            nc.scalar.copy(dst, src)
                bds.append(bd)

            # Big: out rows pair q = bd_q.T @ img_q, in 512-col chunks.
            # PSUM -> bf16 staging (vector/scalar alternate), one output
            # DMA per q on the Sync ring (idle after the input issues).
            for q in range(2):
                ot = opool.tile([128, JS], BF16, tag=f"ot{q}")
                for k in range(4):
                    ps = bpsum.tile([128, 512], F32, tag="mm_ps")
                    nc.tensor.matmul(
                        ps[:], bds[q][:],
                        imgs[:, JS * q + 512 * k:JS * q + 512 * k + 512],
                        start=True, stop=True,
                    )
                    dst = ot[:, 512 * k:512 * k + 512]
                    if k % 2 == 0:
                        nc.vector.tensor_copy(dst, ps[:])
                    else:
                        nc.scalar.copy(dst, ps[:])
                nc.sync.dma_start(
                    out_d.ap()[:, JS * q:JS * (q + 1)], ot[:])

    nc.compile()
    return nc


def _prep_inputs(fpsf, fimg, Wq, Wk):
    fpsf = np.ascontiguousarray(fpsf, dtype=np.float32)
    fimg = np.ascontiguousarray(fimg, dtype=np.float32)
    Wq = np.ascontiguousarray(Wq, dtype=np.float32)
    Wk = np.ascontiguousarray(Wk, dtype=np.float32)

    WqF = np.empty((64, 4100), NPBF16)
    WqF[:, 0:4] = fpsf.T.astype(NPBF16)
    WqF[:, 4:4100] = Wq.T.astype(NPBF16)

    # Wk_nat[d, 64h+c] = Wk[64h+d, c]
    Wk_nat = np.ascontiguousarray(
        Wk.reshape(64, 64, 64).transpose(1, 0, 2).reshape(64, 4096)
    ).astype(NPBF16)
    Wk2 = np.empty((128, 2048), NPBF16)
    Wk2[0:64, :] = Wk_nat[:, 0:2048]
    Wk2[64:128, :] = Wk_nat[:, 2048:4096]

    fimg_f = fimg.reshape(B, C, HW).astype(NPBF16)
    in_maps = []
    for i in range(N_CORES):
        sh = np.ascontiguousarray(
            fimg_f[:, :, JS * i:JS * (i + 1)]).reshape(2, 128, JS)
        sh = np.ascontiguousarray(
            sh.transpose(1, 0, 2).reshape(128, 2 * JS))
        in_maps.append({
            "fimg_s": sh,
            "WqF": WqF,
            "Wk2": Wk2,
        })
    return in_maps


def kernel(fpsf, fimg, Wq, Wk):
    global _compiled
    if _compiled is None:
        _compiled = _build()
    nc = _compiled

    in_maps = _prep_inputs(fpsf, fimg, Wq, Wk)
    res = run_bass_kernel_spmd(nc, in_maps, core_ids=list(range(N_CORES)))

    out = np.empty((B, HEADS, HW), dtype=np.float32)
    for i in range(N_CORES):
        o = res.results[i]["out"]  # [128, 2*JS] bf16
        o = o.reshape(128, 2, JS).transpose(1, 0, 2).reshape(B, HEADS, JS)
        out[:, :, JS * i:JS * (i + 1)] = o.astype(np.float32)
    return out.reshape(B, C, H, W)


if __name__ == "__main__":
    rng = np.random.default_rng(0)
    ins = {
        "fpsf": rng.standard_normal((B, C), dtype=np.float32),
        "fimg": rng.standard_normal((B, C, H, W), dtype=np.float32),
        "Wq": (rng.standard_normal((4096, C), dtype=np.float32) * 0.05),
        "Wk": (rng.standard_normal((4096, C), dtype=np.float32) * 0.05),
    }
    out = kernel(**ins)
    print("out", out.shape, out.dtype, float(np.abs(out).max()))


# revision 19
# speedup vs baseline: 1.4028x; 1.0382x over previous
"""Trainium2 Bass kernel for nn_CrossAttention (single-query cross attention).

Reference computation (B=4, C=64, H=W=128, heads h=64, dim_head d=64,
inner=4096, HW=16384):
    x[b, j, c]   = fimg[b, c, j]                       (j indexes H*W)
    q[b, h, d]   = sum_e fpsf[b, e] Wq[h*64+d, e]
    k[b, j, h, d]= sum_c x[b, j, c] Wk[h*64+d, c]
    out[b, h, j] = scale * sum_d q[b,h,d] k[b,j,h,d]

Because there is a single query per (batch, head), the attention collapses:
    W2[b, h, c]  = scale * sum_d q[b,h,d] Wk[h*64+d, c]      (tiny)
    out[b, h, j] = sum_c W2[b,h,c] fimg[b, c, j]
a 64x FLOP reduction vs materializing k.

Sharding: the j (H*W = 16384) axis is split across the 8 cores (2048 each).
Every core redundantly computes W2 (it needs all heads for its output).

v2 changes vs the 29.0us baseline (profile-driven):
  - DMA issue count 11 -> 6. Each HWDGE DMA_DIRECT2D costs ~650ns of
    serial Sync-engine time; the baseline spent 7.6us just issuing DMAs.
  - DMA priority order: weights (wqf, wkbd) before fimg. The baseline
    interleaved them, so step B stalled ~7us waiting for Wk_bd while
    fimg (not needed until the big matmuls) hogged the DMA engines.
  - Output staged and DMA'd as bf16 (host casts back to f32): halves
    output HBM traffic 2MB -> 1MB per core. Adds ~1e-3 rel err.
  - fimg packed as one [128, 4096] tensor (1 DMA); out as [128, 4096].

Device layouts (prepared host-side; host does LAYOUT only, no math):
  WqF    [64, 4100] bf16: cols 0:4 = fpsf.T, cols 4: = Wq.T  (kept at
                         base partition 0: bf16 matmuls with operands at
                         partition offset 64 crash TRN2)
  Wk_bd  [128, 4096] bf16: per head-pair p, cols 128p..128p+128 hold
                         block-diag [[Wk_{2p}[d,c], 0], [0, Wk_{2p+1}[d,c]]]
                         (128-col stationary keeps compiler FWL active)
  fimg_s [128, 4096] bf16: rows b%2*64+c, cols 2048*(b//2) + local j
  out    [128, 4096] bf16: rows b%2*64+h, cols 2048*(b//2) + local j

Device compute per core:
  A: 32 matmuls  q2T chunk [128, 4] = WqT_chunk.T @ fpsfT
     -> q2T psum [128, 128] with cols 4p+b
  copy: q2T psum -> SBUF bf16 with the attention scale folded in
  B: 32 matmuls  w2 [128, 4] = Wk_bd_p.T @ q2T[:, 4p:4p+4]
     -> w2 psum [128, 128]: rows c + 64*(h%2), cols 4*(h//2)+b
  Assembly: per batch-pair q, block-diag lhsT bd_q [128, 128] (bf16):
     bd_q[64*half + c, 64*half + h] = W2[2q+half, h, c]
  Big: 8 matmuls [128, 512] = bd_q.T @ fimg cols; psum -> bf16 SBUF
     staging [128, 2048] per q (vector/scalar alternate); out DMA per q.
"""

import sys
import types

import numpy as np
import ml_dtypes

# antenv.axon_hooks is absent in this image; bass_utils imports it when
# tracing. Register a minimal stand-in before importing concourse.
if "antenv.axon_hooks" not in sys.modules:
    try:
        import antenv  # noqa: F401

        _hooks = types.ModuleType("antenv.axon_hooks")
        _hooks._hook = None

        def _set_hook(h):
            _hooks._hook = h

        _hooks.set_axon_ntff_profile_hook = _set_hook
        _hooks.get_axon_ntff_profile_hook = lambda: _hooks._hook
        sys.modules["antenv.axon_hooks"] = _hooks
        try:
            from trn_agent_boot.trn_boot import _ntff_profile_via_ctypes

            _set_hook(_ntff_profile_via_ctypes("/opt/axon/libaxon_pjrt.so"))
        except Exception:
            pass
    except ImportError:
        pass

import concourse.bass as bass  # noqa: E402
import concourse.mybir as mybir  # noqa: E402
import concourse.tile as tile  # noqa: E402
from concourse import bacc  # noqa: E402
from concourse.bass_utils import run_bass_kernel_spmd  # noqa: E402

N_CORES = 8
B, C, H, W = 4, 64, 128, 128
HEADS, DIM_HEAD = 64, 64
HW = H * W
JS = HW // N_CORES  # 2048 j-positions per core
SCALE = DIM_HEAD ** -0.5
F32 = mybir.dt.float32
BF16 = mybir.dt.bfloat16
NPBF16 = ml_dtypes.bfloat16

_compiled = None  # cache (nc) across calls


def _build():
    nc = bacc.Bacc("TRN2", target_bir_lowering=False, debug=False,
                   num_devices=N_CORES)

    fimg_d = nc.dram_tensor("fimg_s", [128, 2 * JS], BF16, kind="ExternalInput")
    wqf_d = nc.dram_tensor("WqF", [64, 4100], BF16, kind="ExternalInput")
    wkbd_d = nc.dram_tensor("Wk_bd", [128, 4096], BF16, kind="ExternalInput")
    out_d = nc.dram_tensor("out", [128, 2 * JS], BF16, kind="ExternalOutput")

    with tile.TileContext(nc) as tc:
        with (
            tc.tile_pool(name="weights", bufs=1) as wpool,
            tc.tile_pool(name="img", bufs=1) as ipool,
            tc.tile_pool(name="small_ps", bufs=1, space="PSUM") as spsum,
            tc.tile_pool(name="big_ps", bufs=6, space="PSUM") as bpsum,
            tc.tile_pool(name="ostage", bufs=2) as opool,
        ):
            # Input DMAs in priority order on the Sync HWDGE ring (FIFO
            # per issuing engine): wqf gates step A, wkbd gates step B
            # (split in two so B's first half starts earlier), fimg is
            # only needed by the big matmuls at the end.
            wqf = wpool.tile([64, 4100], BF16, tag="wqf")
            nc.sync.dma_start(wqf[:], wqf_d.ap()[:])
            fpsfT = wqf[:, 0:4]
            wqT = wqf[:, 4:4100]
            wkbd = wpool.tile([128, 4096], BF16, tag="wkbd")
            nc.sync.dma_start(wkbd[:, 0:2048], wkbd_d.ap()[:, 0:2048])
            nc.sync.dma_start(wkbd[:, 2048:4096], wkbd_d.ap()[:, 2048:4096])
            imgs = ipool.tile([128, 2 * JS], BF16, tag="img")
            nc.sync.dma_start(imgs[:], fimg_d.ap()[:])

            # A: q2T[p_row, 4p+b] = q2[b, 128p + p_row] (scale folded
            # into the PSUM->SBUF copy below)
            q2T_ps = spsum.tile([128, 128], F32, tag="q2T_ps")
            for p in range(32):
                nc.tensor.matmul(
                    q2T_ps[:, 4 * p:4 * p + 4],
                    wqT[:, 128 * p:128 * p + 128],
                    fpsfT,
                    start=True, stop=True,
                )
            q2T = wpool.tile([128, 128], BF16, tag="q2T")
            nc.vector.tensor_scalar_mul(q2T[:], q2T_ps[:], SCALE)

            # B: w2[c + 64*(h%2), 4*(h//2)+b] = W2[b, h, c] (scaled)
            w2_ps = spsum.tile([128, 128], F32, tag="w2_ps")
            for p in range(32):
                nc.tensor.matmul(
                    w2_ps[:, 4 * p:4 * p + 4],
                    wkbd[:, 128 * p:128 * p + 128],
                    q2T[:, 4 * p:4 * p + 4],
                    start=True, stop=True,
                )

            # Assembly: bd_q[64*half + c, 64*half + h] = W2[2q+half, h, c]
            bds = []
            for q in range(2):
                bd = wpool.tile([128, 128], BF16, tag=f"bd{q}")
                nc.vector.memset(bd[:], 0.0)
                for half in range(2):
                    b = 2 * q + half
                    for parity in range(2):
                        dst = bd[64 * half:64 * half + 64,
                                 64 * half + parity:64 * half + 64:2]
                        src = w2_ps[64 * parity:64 * parity + 64, b:128:4]
                        nc.vector.tensor_copy(dst, src)
                bds.append(bd)

            # Big: out rows pair q = bd_q.T @ img_q, in 512-col chunks.
            # PSUM -> bf16 staging (vector/scalar alternate), one output
            # DMA per q on the Sync ring (idle after the input issues).
            for q in range(2):
                ot = opool.tile([128, JS], BF16, tag=f"ot{q}")
                for k in range(4):
                    ps = bpsum.tile([128, 512], F32, tag="mm_ps")
                    nc.tensor.matmul(
                        ps[:], bds[q][:],
                        imgs[:, JS * q + 512 * k:JS * q + 512 * k + 512],
                        start=True, stop=True,
                    )
                    dst = ot[:, 512 * k:512 * k + 512]
                    if k % 2 == 0:
                        nc.vector.tensor_copy(dst, ps[:])
                    else:
                        nc.scalar.copy(dst, ps[:])
                nc.sync.dma_start(
                    out_d.ap()[:, JS * q:JS * (q + 1)], ot[:])

    nc.compile()
    return nc


def _prep_inputs(fpsf, fimg, Wq, Wk):
    fpsf = np.ascontiguousarray(fpsf, dtype=np.float32)
    fimg = np.ascontiguousarray(fimg, dtype=np.float32)
    Wq = np.ascontiguousarray(Wq, dtype=np.float32)
    Wk = np.ascontiguousarray(Wk, dtype=np.float32)

    WqF = np.empty((64, 4100), NPBF16)
    WqF[:, 0:4] = fpsf.T.astype(NPBF16)
    WqF[:, 4:4100] = Wq.T.astype(NPBF16)

    Wk3 = Wk.reshape(64, 64, 64)  # [h, d, c]
    bd = np.zeros((128, 32, 128), np.float32)
    bd[0:64, :, 0:64] = Wk3[0::2].transpose(1, 0, 2)   # [d, pair, c]
    bd[64:128, :, 64:128] = Wk3[1::2].transpose(1, 0, 2)
    Wk_bd = np.ascontiguousarray(bd.reshape(128, 4096)).astype(NPBF16)

    fimg_f = fimg.reshape(B, C, HW).astype(NPBF16)
    in_maps = []
    for i in range(N_CORES):
        sh = np.ascontiguousarray(
            fimg_f[:, :, JS * i:JS * (i + 1)]).reshape(2, 128, JS)
        sh = np.ascontiguousarray(
            sh.transpose(1, 0, 2).reshape(128, 2 * JS))
        in_maps.append({
            "fimg_s": sh,
            "WqF": WqF,
            "Wk_bd": Wk_bd,
        })
    return in_maps


def kernel(fpsf, fimg, Wq, Wk):
    global _compiled
    if _compiled is None:
        _compiled = _build()
    nc = _compiled

    in_maps = _prep_inputs(fpsf, fimg, Wq, Wk)
    res = run_bass_kernel_spmd(nc, in_maps, core_ids=list(range(N_CORES)))

    out = np.empty((B, HEADS, HW), dtype=np.float32)
    for i in range(N_CORES):
        o = res.results[i]["out"]  # [128, 2*JS] bf16
        o = o.reshape(128, 2, JS).transpose(1, 0, 2).reshape(B, HEADS, JS)
        out[:, :, JS * i:JS * (i + 1)] = o.astype(np.float32)
    return out.reshape(B, C, H, W)


if __name__ == "__main__":
    rng = np.random.default_rng(0)
    ins = {
        "fpsf": rng.standard_normal((B, C), dtype=np.float32),
        "fimg": rng.standard_normal((B, C, H, W), dtype=np.float32),
        "Wq": (rng.standard_normal((4096, C), dtype=np.float32) * 0.05),
        "Wk": (rng.standard_normal((4096, C), dtype=np.float32) * 0.05),
    }
    out = kernel(**ins)
    print("out", out.shape, out.dtype, float(np.abs(out).max()))
